# revision 48
# baseline (speedup 1.0000x reference)
"""Trainium2 Bass kernel for nn_DetectionLoss (B=128, N=1024, MAX_T=64, 80 classes).

Contract: kernel(**inputs) takes FULL inputs {preds: (128,1024,85) f32,
targets: (128,64,5) f32} and returns the FULL scalar output (f32 (),
mean of per-sample losses), computed data-parallel on 8 NeuronCores
(16 samples per core).

Layout/optimizations vs the original baseline (412us -> 245us):
- reciprocal via reciprocal_approx_fast (51-ULP, ~5x faster than DVE recip)
- shifted iou domain: ioup1 = (iou+1)*valid  (thresholds 0.5 -> 1.5),
  removes the separate -1 masking pass
- pair loop runs entirely on Vector: concurrent GpSimd ucode ops stall
  DVE ~2.5x (shared-SBUF contention), so a V-pure loop is faster than a
  "balanced" V/G split
- matched-target gather on TensorE: one-hot(fim) per chunk -> PE
  transpose -> bf16 one-hot^T -> 8 tiny matmuls against j-partitioned
  targets; replaces the big DVE one-hot multiply+reduce of the baseline
- intersection chain (W/relu/inter) in bf16 for 2x DVE tensor_tensor
  (validated: final rel err ~2e-3 vs 2e-2 gate)
- CE via pickexp = sum(onehot * exp(logits)) in bf16 with a pairwise
  tree-add before the 1x-mode reduce; ce = ln(sumexp) - ln(pickexp)
- BT5 target broadcast built in quarters, interleaved with the first
  pair samples to cut startup idle
"""
import numpy as np

import concourse.bass as bass
import concourse.bacc as bacc
import concourse.mybir as mybir
import concourse.tile as tile
from contextlib import ExitStack

f32 = mybir.dt.float32
bf16d = mybir.dt.bfloat16
i32 = mybir.dt.int32
AF = mybir.ActivationFunctionType
ALU = mybir.AluOpType
AX = mybir.AxisListType

# problem constants (hardcoded per spec)
B, N, MAX_T, PD = 128, 1024, 64, 85
NCLS = 79              # logits are pred[:, 6:85]
NCORES = 8
S = B // NCORES        # 16 samples per core
P = 128                # partitions
RCH = N // P           # 8 chunks (preds per partition per sample)

CE_G = 4               # samples per CE-phase op


def build_kernel(nc):
    preds_d = nc.dram_tensor("preds", [S, N, PD], f32, kind="ExternalInput")
    tgts_d = nc.dram_tensor("tgts", [S, MAX_T, 5], f32, kind="ExternalInput")
    loss_d = nc.dram_tensor("loss", [1, S], f32, kind="ExternalOutput")

    with tile.TileContext(nc) as tc, ExitStack() as ctx:
        sb = ctx.enter_context(tc.tile_pool(name="sb", bufs=1))
        sc2 = ctx.enter_context(tc.tile_pool(name="sc2", bufs=1))
        ps = ctx.enter_context(tc.tile_pool(name="ps", bufs=2, space="PSUM"))
        pst = ctx.enter_context(tc.tile_pool(name="pst", bufs=1, space="PSUM"))

        # ---------- constants ----------
        ones_col = sb.tile([1, P], f32, tag="ones_col")       # lhsT (K=1, M<=128)
        nc.vector.memset(ones_col[:], 1.0)
        iotn_i = sb.tile([P, MAX_T], i32, tag="iotn_i")
        nc.gpsimd.iota(iotn_i[:], pattern=[[1, MAX_T]], base=-MAX_T, channel_multiplier=0)
        IOTN = sb.tile([P, MAX_T], f32, tag="iotn")           # j - 64 per row
        nc.vector.tensor_copy(IOTN[:], iotn_i[:])
        iot79_i = sb.tile([P, NCLS], i32, tag="iot79_i")
        nc.gpsimd.iota(iot79_i[:], pattern=[[1, NCLS]], base=0, channel_multiplier=0)
        IOTA79H = sb.tile([P, NCLS], bf16d, tag="iota79h")
        nc.vector.tensor_copy(IOTA79H[:], iot79_i[:])
        idn_i = sb.tile([P, P], i32, tag="idn_i")
        nc.gpsimd.iota(idn_i[:], pattern=[[1, P]], base=0, channel_multiplier=-1)
        IDENT = sb.tile([P, P], f32, tag="ident")
        nc.vector.tensor_scalar(IDENT[:], idn_i[:], 0, None, op0=ALU.is_equal)
        # 256B pad: keeps downstream tile addresses at offsets where the pair
        # loop measured fastest (op times are sensitive to SBUF placement)
        PADT = sb.tile([P, P], bf16d, tag="padt")
        nc.vector.tensor_copy(PADT[:], IDENT[:])
        # preload activation tables while DMAs are in flight (lazy loads
        # otherwise land on cross-engine critical paths at phase transitions)
        WARM = sb.tile([P, 8], f32, tag="warm")
        nc.scalar.activation(WARM[:], IDENT[:, 0:8], AF.Ln, bias=1.0)
        nc.scalar.activation(WARM[:], IDENT[:, 0:8], AF.Relu)

        # ---------- loads (targets first: they feed the pair-loop prologue) ----------
        TROW = sb.tile([1, S, MAX_T, 5], f32, tag="trow")
        nc.sync.dma_start(TROW[:], tgts_d[:].rearrange("s t c -> (s t c)").unsqueeze(0))
        # first pair-loop samples' preds, then j-partitioned targets, then the rest
        PRED = sb.tile([P, S, RCH, PD], f32, tag="pred")      # 43.5 KB/part
        for s in range(4):
            nc.sync.dma_start(PRED[:, s], preds_d[s].rearrange("(p r) q -> p r q", p=P))
        TGTJ = sb.tile([P, S, 5], f32, tag="tgtj")
        for s in range(S):
            nc.sync.dma_start(TGTJ[0:MAX_T, s], tgts_d[s])
        TGTJH = sb.tile([P, S, 8], bf16d, tag="tgtjh")        # rhs padded to 8 cols
        nc.vector.memset(TGTJH[:], 0.0)
        nc.scalar.copy(TGTJH[0:MAX_T, :, 0:5], TGTJ[0:MAX_T])
        for s in range(4, S):
            nc.sync.dma_start(PRED[:, s], preds_d[s].rearrange("(p r) q -> p r q", p=P))

        # BT5[p, q, s, j] = targets[s, j, q] via TensorE ones-matmul broadcast,
        # built per 8-sample half so the pair loop can start after the first half
        BT5 = sb.tile([P, 5, S, MAX_T], f32, tag="bt5")       # 20 KB/part
        A2 = sb.tile([P, S, MAX_T], f32, tag="a2")
        AT = sc2.tile([P, 4, MAX_T], f32, tag="attmp")
        VB = sb.tile([P, S, MAX_T], f32, tag="vb")            # valid mask 1/0
        PA = sb.tile([P, S, RCH], f32, tag="pa")
        PW = sb.tile([P, S, RCH], f32, tag="pw")
        PH = sb.tile([P, S, RCH], f32, tag="ph")

        def build_half(h):
            hs = slice(h * 4, (h + 1) * 4)
            for q in (2, 0, 3, 1, 4):                         # A2 inputs first
                rhs = TROW[0:1, hs, :, q]                     # (1, 4, 64) strided
                bt_ps = ps.tile([P, 4 * MAX_T], f32, tag="bt_ps")
                nc.tensor.matmul(bt_ps[:], ones_col[:], rhs, start=True, stop=True)
                nc.scalar.copy(BT5[:, q, hs, :], bt_ps[:])
            nc.vector.tensor_tensor(A2[:, hs], BT5[:, 2, hs], BT5[:, 0, hs], op=ALU.subtract)
            nc.vector.tensor_tensor(AT[:], BT5[:, 3, hs], BT5[:, 1, hs], op=ALU.subtract)
            nc.vector.tensor_tensor(A2[:, hs], A2[:, hs], AT[:], op=ALU.mult)
            nc.vector.tensor_scalar(VB[:, hs], BT5[:, 4, hs], 0.0, None, op0=ALU.is_ge)
            nc.vector.tensor_tensor(PW[:, hs], PRED[:, hs, :, 2], PRED[:, hs, :, 0], op=ALU.subtract)
            nc.vector.tensor_tensor(PH[:, hs], PRED[:, hs, :, 3], PRED[:, hs, :, 1], op=ALU.subtract)
            nc.vector.scalar_tensor_tensor(PA[:, hs], PW[:, hs], 1e-6, PH[:, hs], ALU.bypass, ALU.mult)
            nc.vector.tensor_scalar(PA[:, hs], PA[:, hs], 1e-6, None, op0=ALU.add)

        build_half(0)

        # ---------- per-pred accumulators ----------
        BEST = sb.tile([P, S, RCH], f32, tag="best")
        FIM = sb.tile([P, S, RCH], f32, tag="fim")
        MTALL = sb.tile([P, S, RCH, 5], f32, tag="mtall")
        SUMEXP = sb.tile([P, S, RCH], f32, tag="sumexp")
        PICKE = sb.tile([P, S, RCH], f32, tag="picke")
        SL1S = sb.tile([P, S, RCH], f32, tag="sl1s")
        FQ = sb.tile([P, 6, S, RCH], f32, tag="fq")

        SH3 = [P, RCH, MAX_T]
        SH4 = [P, 2, RCH, MAX_T]

        def bcast_t(ap64):       # (P, 64) -> (P, RCH, 64)
            return ap64.unsqueeze(1).broadcast_to(SH3)

        def bcast_p(ap8):        # (P, RCH) -> (P, RCH, 64)
            return ap8.unsqueeze(2).broadcast_to(SH3)

        # ---------- pair phase: software-pipelined over samples ----------
        # stage A (I-ops + W + scalar relu) runs one sample ahead so Vector
        # has work queued while ScalarE computes wxr for the previous sample.
        def stageA(s):
            I2 = sc2.tile(SH4, bf16d, tag="i2", bufs=2)
            nc.vector.scalar_tensor_tensor(I2[:, 0], bcast_t(BT5[:, 2, s]), 0.0, bcast_p(PRED[:, s, :, 2]), ALU.bypass, ALU.min)
            nc.vector.scalar_tensor_tensor(I2[:, 1], bcast_t(BT5[:, 3, s]), 0.0, bcast_p(PRED[:, s, :, 3]), ALU.bypass, ALU.min)
            I1 = sc2.tile(SH4, bf16d, tag="i1", bufs=2)
            nc.vector.scalar_tensor_tensor(I1[:, 0], bcast_t(BT5[:, 0, s]), 0.0, bcast_p(PRED[:, s, :, 0]), ALU.bypass, ALU.max)
            nc.vector.scalar_tensor_tensor(I1[:, 1], bcast_t(BT5[:, 1, s]), 0.0, bcast_p(PRED[:, s, :, 1]), ALU.bypass, ALU.max)
            W = sc2.tile(SH4, bf16d, tag="w", bufs=2)
            nc.vector.tensor_tensor(W[:], I2[:], I1[:], op=ALU.subtract)
            wxr = sc2.tile(SH3, bf16d, tag="wxr", bufs=2)
            nc.scalar.activation(wxr[:], W[:, 0], AF.Relu)
            return W, wxr

        # stage B processes a PAIR of samples: the per-sample STT ops write
        # into [P, 2, RCH, MAX_T] slices; the dense TT/reduce/reciprocal ops
        # then run once over the flat pair (halves their 1x-mode init cost).
        SH3P = [P, 2, RCH, MAX_T]

        def stageB2(s0, A0, A1):
            inter = sc2.tile(SH3P, bf16d, tag="inter", bufs=1)
            den1 = sc2.tile(SH3P, f32, tag="den1", bufs=1)
            den = sc2.tile(SH3P, f32, tag="den", bufs=1)
            for k, (W, wxr) in enumerate((A0, A1)):
                s = s0 + k
                nc.vector.scalar_tensor_tensor(inter[:, k], W[:, 1], 0.0, wxr[:], ALU.max, ALU.mult)
                nc.vector.scalar_tensor_tensor(den1[:, k], inter[:, k], -1.0, bcast_t(A2[:, s]), ALU.mult, ALU.add)
                nc.vector.scalar_tensor_tensor(den[:, k], den1[:, k], 0.0, bcast_p(PA[:, s]), ALU.bypass, ALU.add)
            rcp = sc2.tile(SH3P, f32, tag="rcp", bufs=1)
            nc.vector.reciprocal_approx_fast(rcp[:].rearrange("p a r j -> p (a r j)"), den[:].rearrange("p a r j -> p (a r j)"))
            iou = sc2.tile(SH3P, f32, tag="iou", bufs=1)
            nc.vector.tensor_tensor(iou[:], inter[:], rcp[:], op=ALU.mult)
            # shifted domain: ioup1 = (iou + 1) * valid; invalid -> 0
            ioup1 = sc2.tile(SH3P, f32, tag="ioup1", bufs=1)
            for k in range(2):
                nc.vector.scalar_tensor_tensor(ioup1[:, k], iou[:, k], 1.0, bcast_t(VB[:, s0 + k]), ALU.add, ALU.mult)
            nc.vector.tensor_reduce(BEST[:, s0:s0 + 2], ioup1[:], axis=AX.X, op=ALU.max)
            eq = sc2.tile(SH3P, f32, tag="den1", bufs=1)
            for k in range(2):
                nc.vector.scalar_tensor_tensor(eq[:, k], ioup1[:, k], 0.0, bcast_p(BEST[:, s0 + k]), ALU.bypass, ALU.is_equal)
            eqi = sc2.tile(SH3P, f32, tag="eqi", bufs=1)
            iotn_b2 = IOTN[:].unsqueeze(1).unsqueeze(1).broadcast_to(SH3P)
            nc.vector.tensor_tensor(eqi[:], eq[:], iotn_b2, op=ALU.mult)
            nc.vector.tensor_reduce(FIM[:, s0:s0 + 2], eqi[:], axis=AX.X, op=ALU.min)

            # ---- matched-target gather on TensorE ----
            oh = sc2.tile(SH3P, f32, tag="oh", bufs=1)
            for k in range(2):
                s = s0 + k
                nc.vector.scalar_tensor_tensor(oh[:, k], bcast_t(IOTN[:]), 0.0, bcast_p(FIM[:, s]), ALU.bypass, ALU.is_equal)
                ohtp = pst.tile([MAX_T, RCH, P], f32, tag="ohtp", bufs=1)
                for r in range(RCH):
                    nc.tensor.transpose(ohtp[:, r], oh[:, k, r, :], IDENT[:])
                OHTS = sc2.tile([MAX_T, RCH, P], bf16d, tag="ohts", bufs=1)
                nc.scalar.copy(OHTS[:], ohtp[:])
                mtp = pst.tile([P, RCH, 8], f32, tag="mtp", bufs=1)
                for r in range(RCH):
                    nc.tensor.matmul(mtp[:, r], OHTS[:, r, :], TGTJH[0:MAX_T, s], start=True, stop=True)
                nc.scalar.copy(MTALL[:, s], mtp[:, :, 0:5])

        # CE block for a 4-sample group: exp + group sums + picked expo.
        # Emitted inside the pair loop as soon as the group's gathers are done,
        # so the ScalarE exp overlaps pair compute instead of stalling V at a
        # phase boundary.
        LBLH = sb.tile([P, S, RCH], bf16d, tag="lblh")
        SHC = [P, CE_G, RCH, NCLS]

        def ce_block(h):
            sl = slice(CE_G * h, CE_G * (h + 1))
            nc.vector.tensor_scalar(LBLH[:, sl], MTALL[:, sl, :, 4], 0.0, None, op0=ALU.max)
            e2h = sc2.tile(SHC, bf16d, tag="e2h", bufs=2)
            nc.scalar.activation(e2h[:], PRED[:, sl, :, 6:], AF.Exp)
            u2 = sc2.tile([P, CE_G, RCH, 39], bf16d, tag="u2", bufs=1)
            nc.vector.tensor_tensor(u2[:], e2h[:, :, :, 0:39], e2h[:, :, :, 39:78], op=ALU.add)
            nc.vector.tensor_reduce(SUMEXP[:, sl], u2[:], axis=AX.X, op=ALU.add)
            nc.vector.tensor_tensor(SUMEXP[:, sl], SUMEXP[:, sl], e2h[:, :, :, 78], op=ALU.add)
            ohc = sc2.tile(SHC, bf16d, tag="ohc", bufs=1)
            iot79b = IOTA79H[:].unsqueeze(1).unsqueeze(1).broadcast_to(SHC)
            lblb = LBLH[:, sl].unsqueeze(3).broadcast_to(SHC)
            nc.vector.tensor_tensor(ohc[:], iot79b, lblb, op=ALU.is_equal)
            pk2 = sc2.tile(SHC, bf16d, tag="pk2", bufs=1)
            nc.vector.tensor_tensor(pk2[:], ohc[:], e2h[:], op=ALU.mult)
            v2 = sc2.tile([P, CE_G, RCH, 39], bf16d, tag="u2", bufs=1)
            nc.vector.tensor_tensor(v2[:], pk2[:, :, :, 0:39], pk2[:, :, :, 39:78], op=ALU.add)
            nc.vector.tensor_reduce(PICKE[:, sl], v2[:], axis=AX.X, op=ALU.add)
            nc.vector.tensor_tensor(PICKE[:, sl], PICKE[:, sl], pk2[:, :, :, 78], op=ALU.add)

        pA = stageA(0)
        pB = stageA(1)
        for s0 in range(0, S, 2):
            if s0 + 2 < S:
                if (s0 + 2) % 4 == 0:
                    build_half((s0 + 2) // 4)
                nA = stageA(s0 + 2)
                nB = stageA(s0 + 3)
            stageB2(s0, pA, pB)
            if s0 + 2 < S:
                pA, pB = nA, nB

        for h in range(S // CE_G):
            ce_block(h)

        # ce = ln(sumexp) - ln(pickexp)
        LSE = sb.tile([P, S, RCH], f32, tag="lse")
        nc.scalar.activation(LSE[:], SUMEXP[:], AF.Ln)
        LPK = sb.tile([P, S, RCH], f32, tag="lpk")
        nc.scalar.activation(LPK[:], PICKE[:], AF.Ln)
        CE = sb.tile([P, S, RCH], f32, tag="ce")
        nc.vector.tensor_tensor(CE[:], LSE[:], LPK[:], op=ALU.subtract)

        # ---------- smooth L1 (all samples, bf16 for 2x/4x DVE modes) ----------
        DD = sb.tile([P, S, RCH, 4], bf16d, tag="dd")
        nc.vector.tensor_tensor(DD[:], PRED[:, :, :, 0:4], MTALL[:, :, :, 0:4], op=ALU.subtract)
        AD = sb.tile([P, S, RCH, 4], bf16d, tag="ad")
        nc.vector.scalar_tensor_tensor(AD[:], DD[:], -1.0, DD[:], ALU.mult, ALU.max)
        TM = sb.tile([P, S, RCH, 4], bf16d, tag="tm")
        nc.vector.tensor_scalar(TM[:], AD[:], 1.0, None, op0=ALU.min)
        UU = sb.tile([P, S, RCH, 4], bf16d, tag="uu")
        nc.vector.scalar_tensor_tensor(UU[:], TM[:], -0.5, AD[:], ALU.mult, ALU.add)
        SL1 = sb.tile([P, S, RCH, 4], bf16d, tag="sl1")
        nc.vector.tensor_tensor(SL1[:], TM[:], UU[:], op=ALU.mult)
        nc.vector.tensor_reduce(SL1S[:], SL1[:], axis=AX.X, op=ALU.add)

        # ---------- conf softplus (all samples) ----------
        CF = PRED[:, :, :, 4]
        AXC = sb.tile([P, S, RCH], f32, tag="axc")
        nc.vector.scalar_tensor_tensor(AXC[:], CF, -1.0, CF, ALU.mult, ALU.max)
        EN = sb.tile([P, S, RCH], f32, tag="en")
        nc.scalar.activation(EN[:], AXC[:], AF.Exp, scale=-1.0)
        L1 = sb.tile([P, S, RCH], f32, tag="l1")
        nc.scalar.activation(L1[:], EN[:], AF.Ln, bias=1.0)
        MX0 = sb.tile([P, S, RCH], f32, tag="mx0")
        nc.vector.tensor_scalar(MX0[:], CF, 0.0, None, op0=ALU.max)
        # SPP -> FQ[:,5]; SPN separate
        nc.vector.tensor_tensor(FQ[:, 5], L1[:], MX0[:], op=ALU.add)
        SPN = sb.tile([P, S, RCH], f32, tag="spn")
        nc.vector.tensor_tensor(SPN[:], FQ[:, 5], CF, op=ALU.subtract)

        # ---------- match mask (shifted domain: thresholds 0.5 -> 1.5) ----------
        BESTS16 = sb.tile([P, S], f32, tag="bests16")
        nc.vector.tensor_reduce(BESTS16[:], BEST[:], axis=AX.X, op=ALU.max)
        trb = pst.tile([S, P], f32, tag="tp128")
        nc.tensor.transpose(trb[:], BESTS16[:], IDENT[:])
        TB = sb.tile([S, P], f32, tag="tb")
        nc.scalar.copy(TB[:], trb[:])
        GMAX16 = sb.tile([S, 1], f32, tag="gmax16")
        nc.vector.tensor_reduce(GMAX16[:], TB[:], axis=AX.X, op=ALU.max)
        EQT = sb.tile([S, P], f32, tag="eqt")
        nc.vector.tensor_scalar(EQT[:], TB[:], GMAX16[:], None, op0=ALU.is_equal)
        NAFT = sb.tile([S, 1], f32, tag="naft")
        nc.vector.tensor_scalar(NAFT[:], GMAX16[:], 1.5, None, op0=ALU.is_le)
        NF128 = sb.tile([S, P], f32, tag="nf128")
        nc.vector.tensor_scalar(NF128[:], TB[:], 0.0, NAFT[:], op0=ALU.mult, op1=ALU.add)
        teqc = pst.tile([P, S], f32, tag="tp128")
        nc.tensor.transpose(teqc[:], EQT[:], IDENT[:S, :S])
        EQC = sb.tile([P, S], f32, tag="eqc")
        nc.scalar.copy(EQC[:], teqc[:])
        tnaf = pst.tile([P, S], f32, tag="tp128")
        nc.tensor.transpose(tnaf[:], NF128[:], IDENT[:S, :S])
        NAFC = sb.tile([P, S], f32, tag="nafc")
        nc.scalar.copy(NAFC[:], tnaf[:])

        MR = sb.tile([P, S, RCH], f32, tag="mr")
        nc.vector.tensor_scalar(MR[:], BEST[:], 1.5, None, op0=ALU.is_gt)
        EQB = sb.tile([P, S, RCH], f32, tag="eqb")
        nc.vector.tensor_tensor(EQB[:], BEST[:], BESTS16[:].unsqueeze(2).broadcast_to([P, S, RCH]), op=ALU.is_equal)
        EQG = sb.tile([P, S, RCH], f32, tag="eqg")
        nc.vector.tensor_tensor(EQG[:], EQB[:], EQC[:].unsqueeze(2).broadcast_to([P, S, RCH]), op=ALU.mult)
        M2 = sb.tile([P, S, RCH], f32, tag="m2")
        nc.vector.tensor_tensor(M2[:], EQG[:], NAFC[:].unsqueeze(2).broadcast_to([P, S, RCH]), op=ALU.mult)
        # M -> FQ[:,0]
        nc.vector.tensor_tensor(FQ[:, 0], MR[:], M2[:], op=ALU.add)

        # ---------- weighted sums into FQ ----------
        nc.vector.tensor_tensor(FQ[:, 1], FQ[:, 0], SL1S[:], op=ALU.mult)
        nc.vector.tensor_tensor(FQ[:, 2], FQ[:, 0], CE[:], op=ALU.mult)
        nc.vector.tensor_tensor(FQ[:, 3], FQ[:, 0], SPN[:], op=ALU.mult)
        nc.vector.tensor_tensor(FQ[:, 4], FQ[:, 0], FQ[:, 5], op=ALU.mult)

        # ---------- partition reductions via transpose ----------
        RS = sb.tile([P, 6], f32, tag="rs")                   # per (s,r) sums
        for k in range(6):
            tq = pst.tile([P, P], f32, tag="tp128")
            nc.tensor.transpose(tq[:], FQ[:, k].rearrange("p s r -> p (s r)"), IDENT[:])
            nc.vector.tensor_reduce(RS[:, k:k + 1], tq[:], axis=AX.X, op=ALU.add)
        trs = pst.tile([6, P], f32, tag="tp128")
        nc.tensor.transpose(trs[:], RS[:], IDENT[:])
        RQ = sb.tile([6, S], f32, tag="rq")                   # per (quantity, sample)
        nc.vector.tensor_reduce(RQ[:], trs[:].rearrange("q (s r) -> q s r", s=S), axis=AX.X, op=ALU.add)
        tf = pst.tile([S, 6], f32, tag="tpsm")
        nc.tensor.transpose(tf[:], RQ[:], IDENT[:6, :6])
        F16 = sb.tile([S, 6], f32, tag="f16")
        nc.scalar.copy(F16[:], tf[:])

        # kv per sample: count of valid targets
        KVC = sb.tile([P, S], f32, tag="kvc")
        nc.vector.tensor_reduce(KVC[:], VB[:], axis=AX.X, op=ALU.add)
        tkv = pst.tile([S, P], f32, tag="tp128")
        nc.tensor.transpose(tkv[:], KVC[:], IDENT[:])
        KV16 = sb.tile([S, 1], f32, tag="kv16")
        nc.vector.tensor_reduce(KV16[:], tkv[:], axis=AX.X, op=ALU.max)

        # ---------- final scalar assembly (partition = sample) ----------
        mcnt = F16[:, 0:1]; bbox_n = F16[:, 1:2]; cls_n = F16[:, 2:3]
        spn_n = F16[:, 3:4]; spp_m = F16[:, 4:5]; spp_all = F16[:, 5:6]

        def t16(tag):
            return sb.tile([S, 1], f32, tag=tag, name=tag)

        d4 = t16("d4"); nc.vector.tensor_scalar(d4[:], mcnt, 4.0, 1.0, op0=ALU.mult, op1=ALU.max)
        r4 = t16("r4"); nc.vector.reciprocal(r4[:], d4[:])
        bbox = t16("bbox"); nc.vector.tensor_tensor(bbox[:], bbox_n, r4[:], op=ALU.mult)
        d1 = t16("d1"); nc.vector.tensor_scalar(d1[:], mcnt, 1.0, None, op0=ALU.max)
        r1 = t16("r1"); nc.vector.reciprocal(r1[:], d1[:])
        clsl = t16("clsl"); nc.vector.tensor_tensor(clsl[:], cls_n, r1[:], op=ALU.mult)
        confm = t16("confm"); nc.vector.tensor_tensor(confm[:], spn_n, r1[:], op=ALU.mult)
        ucnt = t16("ucnt"); nc.vector.tensor_scalar(ucnt[:], mcnt, -1.0, float(N), op0=ALU.mult, op1=ALU.add)
        du = t16("du"); nc.vector.tensor_scalar(du[:], ucnt[:], 1.0, None, op0=ALU.max)
        ru = t16("ru"); nc.vector.reciprocal(ru[:], du[:])
        cun = t16("cun"); nc.vector.tensor_tensor(cun[:], spp_all, spp_m, op=ALU.subtract)
        confu = t16("confu"); nc.vector.tensor_tensor(confu[:], cun[:], ru[:], op=ALU.mult)
        csum = t16("csum"); nc.vector.tensor_tensor(csum[:], confm[:], confu[:], op=ALU.add)
        chalf = t16("chalf"); nc.vector.tensor_scalar(chalf[:], csum[:], 0.5, None, op0=ALU.mult)
        ug = t16("ug"); nc.vector.tensor_scalar(ug[:], ucnt[:], 0.0, None, op0=ALU.is_gt)
        ugn = t16("ugn"); nc.vector.tensor_scalar(ugn[:], ucnt[:], 0.0, None, op0=ALU.is_le)
        c1 = t16("c1"); nc.vector.tensor_tensor(c1[:], chalf[:], ug[:], op=ALU.mult)
        c2 = t16("c2"); nc.vector.tensor_tensor(c2[:], confm[:], ugn[:], op=ALU.mult)
        confL = t16("confL"); nc.vector.tensor_tensor(confL[:], c1[:], c2[:], op=ALU.add)
        lv0 = t16("lv0"); nc.vector.tensor_tensor(lv0[:], bbox[:], clsl[:], op=ALU.add)
        lv = t16("lv"); nc.vector.tensor_tensor(lv[:], lv0[:], confL[:], op=ALU.add)
        lnv = t16("lnv"); nc.vector.tensor_scalar(lnv[:], spp_all, 1.0 / float(N), None, op0=ALU.mult)
        kvg = t16("kvg"); nc.vector.tensor_scalar(kvg[:], KV16[:], 0.0, None, op0=ALU.is_gt)
        kvn = t16("kvn"); nc.vector.tensor_scalar(kvn[:], KV16[:], 0.0, None, op0=ALU.is_le)
        lA = t16("lA"); nc.vector.tensor_tensor(lA[:], lv[:], kvg[:], op=ALU.mult)
        lB = t16("lB"); nc.vector.tensor_tensor(lB[:], lnv[:], kvn[:], op=ALU.mult)
        LOSS16 = t16("loss16"); nc.vector.tensor_tensor(LOSS16[:], lA[:], lB[:], op=ALU.add)

        tl = pst.tile([1, S], f32, tag="tpsm")
        nc.tensor.transpose(tl[:], LOSS16[:], IDENT[:S, :S])
        LROW = sb.tile([1, S], f32, tag="lrow")
        nc.scalar.copy(LROW[:], tl[:])
        nc.sync.dma_start(loss_d[:], LROW[:])

    return preds_d, tgts_d, loss_d


_NC_CACHE = {}


def get_nc():
    if "nc" not in _NC_CACHE:
        nc = bacc.Bacc("TRN2", target_bir_lowering=False, debug=False)
        build_kernel(nc)
        nc.compile()
        _NC_CACHE["nc"] = nc
    return _NC_CACHE["nc"]


def kernel(preds: np.ndarray, targets: np.ndarray) -> np.ndarray:
    from concourse.bass_utils import run_bass_kernel_spmd

    nc = get_nc()
    in_maps = []
    for c in range(NCORES):
        in_maps.append({
            "preds": np.ascontiguousarray(preds[c * S:(c + 1) * S], dtype=np.float32),
            "tgts": np.ascontiguousarray(targets[c * S:(c + 1) * S], dtype=np.float32),
        })
    res = run_bass_kernel_spmd(nc, in_maps, core_ids=list(range(NCORES)))
    per_sample = np.concatenate([res.results[c]["loss"].reshape(-1) for c in range(NCORES)])
    return np.float32(per_sample.sum() / B)
